# revision 3
# baseline (speedup 1.0000x reference)
import sys
from contextlib import ExitStack

sys.path.insert(0, "/opt/trn_rl_repo")

import numpy as np
import ml_dtypes

import concourse.bass as bass
import concourse.bacc as bacc
import concourse.mybir as mybir
import concourse.tile as tile
from concourse.bass_utils import run_bass_kernel_spmd
from concourse.masks import make_identity

B, N, D, H, HD = 4, 4096, 1024, 16, 64
NCORES = 8
T = (B * N) // NCORES  # 2048 tokens per core
P = 128
NT = T // P            # 16 token tiles per core
KT = D // P            # 8 contraction tiles
E3 = 3 * D

# Heads [H - H_POOL, H) of the fat elementwise stages run on GpSimd (Pool)
# instead of DVE to balance the two engines.
H_POOL = 3

_CACHE = {}


def _name(t):
    return t.name if hasattr(t, "name") else t.tensor.name


def _build():
    bf = mybir.dt.bfloat16
    f32 = mybir.dt.float32
    nc = bacc.Bacc(None, target_bir_lowering=False)
    names = {}
    with tile.TileContext(nc) as tc:
        with ExitStack() as ctx:
            dram = ctx.enter_context(tc.tile_pool(name="dram", bufs=1, space="DRAM"))
            xT_d = dram.tile([D, T], bf, kind="ExternalInput")
            wq_d = dram.tile([D, E3], bf, kind="ExternalInput")
            wo_d = dram.tile([D, D], bf, kind="ExternalInput")
            out_d = dram.tile([T, D], f32, kind="ExternalOutput")
            names["xT"] = _name(xT_d)
            names["wqkvT"] = _name(wq_d)
            names["woT"] = _name(wo_d)
            names["out"] = _name(out_d)

            consts = ctx.enter_context(tc.tile_pool(name="consts", bufs=1))
            wq_sb = consts.tile([P, KT, E3], bf)
            wo_sb = consts.tile([P, KT, D], bf)
            ident = consts.tile([P, P], bf)
            make_identity(nc, ident)
            nc.sync.dma_start(out=wq_sb[:], in_=wq_d[:].rearrange("(k p) e -> p k e", p=P))
            nc.sync.dma_start(out=wo_sb[:], in_=wo_d[:].rearrange("(k p) e -> p k e", p=P))

            xin = ctx.enter_context(tc.tile_pool(name="xin", bufs=2))
            qkvp = ctx.enter_context(tc.tile_pool(name="qkvp", bufs=2))
            work = ctx.enter_context(tc.tile_pool(name="work", bufs=1))
            smp = ctx.enter_context(tc.tile_pool(name="smp", bufs=1))
            attnp = ctx.enter_context(tc.tile_pool(name="attnp", bufs=2))
            outp = ctx.enter_context(tc.tile_pool(name="outp", bufs=2))
            psum_mm = ctx.enter_context(tc.tile_pool(name="psum_mm", bufs=2, space="PSUM"))
            psum_tr = ctx.enter_context(tc.tile_pool(name="psum_tr", bufs=2, space="PSUM"))
            psum_o = ctx.enter_context(tc.tile_pool(name="psum_o", bufs=1, space="PSUM"))

            HB = H - H_POOL  # heads on DVE; [HB, H) go to GpSimd

            for i in range(NT):
                tsl = bass.ts(i, P)
                # ---- load x tile (transposed) ----
                xt = xin.tile([P, KT, P], bf, tag="xt")
                nc.sync.dma_start(
                    out=xt[:], in_=xT_d[:, tsl].rearrange("(k p) t -> p k t", p=P)
                )

                # ---- QKV projection: 3 passes of 2x512 cols, k-accumulated ----
                qkv = qkvp.tile([P, E3], bf, tag="qkv")
                for grp in range(3):
                    pa = psum_mm.tile([P, 512], f32, tag="mma")
                    pb = psum_mm.tile([P, 512], f32, tag="mmb")
                    for k in range(KT):
                        nc.tensor.matmul(
                            pa[:], xt[:, k, :], wq_sb[:, k, bass.ts(2 * grp, 512)],
                            start=(k == 0), stop=(k == KT - 1),
                        )
                        nc.tensor.matmul(
                            pb[:], xt[:, k, :], wq_sb[:, k, bass.ts(2 * grp + 1, 512)],
                            start=(k == 0), stop=(k == KT - 1),
                        )
                    nc.scalar.copy(qkv[:, bass.ts(2 * grp, 512)], pa[:])
                    nc.scalar.copy(qkv[:, bass.ts(2 * grp + 1, 512)], pb[:])

                # views: q [t,(h,d)], k [t,(g,d)], v [t,(d,g)] (v is d-major)
                qv = qkv[:, 0:D].rearrange("p (h d) -> p h d", d=HD)
                kv = qkv[:, D:2 * D].rearrange("p (g d) -> p g d", d=HD)
                vv = qkv[:, 2 * D:3 * D].rearrange("p (d g) -> p d g", g=H)

                # ---- scores products tmp[t,h,g,d] = q[t,h,d]*k[t,g,d] ----
                tmp = work.tile([P, H, H, HD], bf, tag="tmp")
                q_b = qv[:, :, None, :].broadcast_to((P, H, H, HD))
                k_b = kv[:, None, :, :].broadcast_to((P, H, H, HD))
                if H_POOL:
                    nc.gpsimd.tensor_mul(tmp[:, HB:H], q_b[:, HB:H], k_b[:, HB:H])
                nc.vector.tensor_mul(tmp[:, 0:HB], q_b[:, 0:HB], k_b[:, 0:HB])

                # ---- reduce over d (fold tree) -> scores [t,h,g] f32 ----
                b1 = work.tile([P, H, H, 32], bf, tag="b1")
                b2 = work.tile([P, H, H, 16], bf, tag="b2")
                scores = smp.tile([P, H, H], f32, tag="sc")
                engs = ((nc.vector, 0, HB),)
                if H_POOL:
                    engs = engs + ((nc.gpsimd, HB, H),)
                for (eng, lo, hi) in engs:
                    eng.tensor_add(b1[:, lo:hi], tmp[:, lo:hi, :, 0:32], tmp[:, lo:hi, :, 32:64])
                    eng.tensor_add(b2[:, lo:hi], b1[:, lo:hi, :, 0:16], b1[:, lo:hi, :, 16:32])
                    eng.tensor_add(b1[:, lo:hi, :, 0:8], b2[:, lo:hi, :, 0:8], b2[:, lo:hi, :, 8:16])
                    eng.tensor_add(b2[:, lo:hi, :, 0:4], b1[:, lo:hi, :, 0:4], b1[:, lo:hi, :, 4:8])
                    eng.tensor_add(b1[:, lo:hi, :, 0:2], b2[:, lo:hi, :, 0:2], b2[:, lo:hi, :, 2:4])
                    eng.tensor_add(
                        scores[:, lo:hi, :, None], b1[:, lo:hi, :, 0:1], b1[:, lo:hi, :, 1:2]
                    )

                # ---- softmax over g: w = exp(s/32); den; rec ----
                wexp = smp.tile([P, H, H], bf, tag="we")
                den = smp.tile([P, H], f32, tag="den")
                rec = smp.tile([P, H], f32, tag="rec")
                dn1 = smp.tile([P, H, 8], f32, tag="dn1")
                nc.scalar.activation(
                    wexp[:], scores[:], mybir.ActivationFunctionType.Exp, scale=1.0 / 32.0
                )
                nc.vector.tensor_add(dn1[:], wexp[:, :, 0:8], wexp[:, :, 8:16])
                nc.vector.tensor_add(dn1[:, :, 0:4], dn1[:, :, 0:4], dn1[:, :, 4:8])
                nc.vector.tensor_add(dn1[:, :, 0:2], dn1[:, :, 0:2], dn1[:, :, 2:4])
                nc.vector.tensor_add(den[:, :, None], dn1[:, :, 0:1], dn1[:, :, 1:2])
                nc.vector.reciprocal(rec[:], den[:])

                # ---- wb[t,h,d,g] = w[t,h,g]*rec[t,h] (normalized, d-bcast) ----
                wb = work.tile([P, H, HD, H], bf, tag="wb")
                for h in range(H):
                    nc.vector.tensor_scalar_mul(
                        wb[:, h],
                        wexp[:, h, None, :].broadcast_to((P, HD, H)),
                        rec[:, h:h + 1],
                    )

                # ---- attn products p2[t,h,d,g] = wb * v[t,d,g] (reuse tmp) ----
                p2 = tmp.rearrange("p h g d -> p (h g d)").rearrange(
                    "p (h d g) -> p h d g", h=H, d=HD
                )
                v_b = vv[:, None, :, :].broadcast_to((P, H, HD, H))
                if H_POOL:
                    nc.gpsimd.tensor_mul(p2[:, HB:H], wb[:, HB:H], v_b[:, HB:H])
                nc.vector.tensor_mul(p2[:, 0:HB], wb[:, 0:HB], v_b[:, 0:HB])

                # ---- reduce over g (fold tree) -> attn [t,(h,d)] bf16 ----
                c1 = b1.rearrange("p h g x -> p (h g x)").rearrange(
                    "p (h d y) -> p h d y", h=H, d=HD
                )  # [P,H,HD,8]
                c2 = b2.rearrange("p h g x -> p (h g x)").rearrange(
                    "p (h d y) -> p h d y", h=H, d=HD
                )  # [P,H,HD,4]
                attn = attnp.tile([P, H, HD], bf, tag="attn")
                for (eng, lo, hi) in engs:
                    eng.tensor_add(c1[:, lo:hi], p2[:, lo:hi, :, 0:8], p2[:, lo:hi, :, 8:16])
                    eng.tensor_add(c2[:, lo:hi], c1[:, lo:hi, :, 0:4], c1[:, lo:hi, :, 4:8])
                    eng.tensor_add(c1[:, lo:hi, :, 0:2], c2[:, lo:hi, :, 0:2], c2[:, lo:hi, :, 2:4])
                    eng.tensor_add(
                        attn[:, lo:hi, :, None], c1[:, lo:hi, :, 0:1], c1[:, lo:hi, :, 1:2]
                    )

                # ---- transpose attn -> attnT [(h d), t] blocks ----
                attn_flat = attn.rearrange("p h d -> p (h d)")
                attnT = attnp.tile([P, KT, P], bf, tag="attnT")
                for c in range(KT):
                    pt = psum_tr.tile([P, P], bf, tag="pt")
                    nc.tensor.transpose(pt[:], attn_flat[:, bass.ts(c, P)], ident[:])
                    nc.scalar.copy(attnT[:, c, :], pt[:])

                # ---- output projection ----
                outt = outp.tile([P, D], f32, tag="outt")
                po_a = psum_o.tile([P, 512], f32, tag="poa")
                po_b = psum_o.tile([P, 512], f32, tag="pob")
                for k in range(KT):
                    nc.tensor.matmul(
                        po_a[:], attnT[:, k, :], wo_sb[:, k, 0:512],
                        start=(k == 0), stop=(k == KT - 1),
                    )
                    nc.tensor.matmul(
                        po_b[:], attnT[:, k, :], wo_sb[:, k, 512:1024],
                        start=(k == 0), stop=(k == KT - 1),
                    )
                nc.scalar.copy(outt[:, 0:512], po_a[:])
                nc.scalar.copy(outt[:, 512:1024], po_b[:])
                nc.sync.dma_start(out=out_d[tsl, :], in_=outt[:])
    nc.compile()
    return nc, names


def kernel(x, Wqkv, Wo, bo, trace=False):
    if "nc" not in _CACHE:
        _CACHE["nc"], _CACHE["names"] = _build()
    nc, names = _CACHE["nc"], _CACHE["names"]
    bf = ml_dtypes.bfloat16
    xt = np.ascontiguousarray(
        np.asarray(x, dtype=np.float32).reshape(B * N, D).T
    )  # [D, B*N]
    wqkv = np.asarray(Wqkv, dtype=np.float32)
    # v rows permuted so the projection emits v[t, (d, g)] (d-major)
    wv = wqkv[2 * D:3 * D].reshape(H, HD, D).transpose(1, 0, 2).reshape(D, D)
    wqkv_perm = np.concatenate([wqkv[0:2 * D], wv], axis=0)
    wqkvT = np.ascontiguousarray(wqkv_perm.T).astype(bf)
    woT = np.ascontiguousarray(np.asarray(Wo, dtype=np.float32).T).astype(bf)
    in_maps = []
    for c in range(NCORES):
        shard = np.ascontiguousarray(xt[:, c * T:(c + 1) * T]).astype(bf)
        in_maps.append(
            {names["xT"]: shard, names["wqkvT"]: wqkvT, names["woT"]: woT}
        )
    res = run_bass_kernel_spmd(
        nc, in_maps, core_ids=list(range(NCORES)), trace=trace
    )
    shards = [res.results[c][names["out"]] for c in range(NCORES)]
    out = np.concatenate(shards, axis=0).reshape(B, N, D).astype(np.float32)
    out = out + np.asarray(bo, dtype=np.float32)[None, None, :]
    if trace:
        return out, res
    return out


# revision 5
# speedup vs baseline: 1.7309x; 1.7309x over previous
import sys
from contextlib import ExitStack

sys.path.insert(0, "/opt/trn_rl_repo")

import numpy as np
import ml_dtypes

import concourse.bass as bass
import concourse.bacc as bacc
import concourse.mybir as mybir
import concourse.tile as tile
from concourse.bass_utils import run_bass_kernel_spmd
from concourse.masks import make_identity

B, N, D, H, HD = 4, 4096, 1024, 16, 64
NCORES = 8
T = (B * N) // NCORES  # 2048 tokens per core
P = 128
NT = T // P            # 16 token tiles per core
KT = D // P            # 8 contraction tiles
E3 = 3 * D
HH = H * H             # 256

_CACHE = {}


def _name(t):
    return t.name if hasattr(t, "name") else t.tensor.name


def _build():
    bf = mybir.dt.bfloat16
    f32 = mybir.dt.float32
    nc = bacc.Bacc(None, target_bir_lowering=False)
    names = {}
    V = None
    with tile.TileContext(nc) as tc:
        with ExitStack() as ctx:
            dram = ctx.enter_context(tc.tile_pool(name="dram", bufs=1, space="DRAM"))
            xT_d = dram.tile([D, T], bf, kind="ExternalInput")
            wq_d = dram.tile([D, E3], bf, kind="ExternalInput")
            wo_d = dram.tile([D, D], bf, kind="ExternalInput")
            out_d = dram.tile([T, D], f32, kind="ExternalOutput")
            names["xT"] = _name(xT_d)
            names["wqkvT"] = _name(wq_d)
            names["woT"] = _name(wo_d)
            names["out"] = _name(out_d)

            consts = ctx.enter_context(tc.tile_pool(name="consts", bufs=1))
            wq_sb = consts.tile([P, KT, E3], bf)
            wo_sb = consts.tile([P, KT, D], bf)
            ident = consts.tile([P, P], bf)
            make_identity(nc, ident)
            nc.sync.dma_start(out=wq_sb[:], in_=wq_d[:].rearrange("(k p) e -> p k e", p=P))
            nc.sync.dma_start(out=wo_sb[:], in_=wo_d[:].rearrange("(k p) e -> p k e", p=P))

            xin = ctx.enter_context(tc.tile_pool(name="xin", bufs=2))
            qkvp = ctx.enter_context(tc.tile_pool(name="qkvp", bufs=2))
            work = ctx.enter_context(tc.tile_pool(name="work", bufs=1))
            smp = ctx.enter_context(tc.tile_pool(name="smp", bufs=1))
            attnp = ctx.enter_context(tc.tile_pool(name="attnp", bufs=2))
            outp = ctx.enter_context(tc.tile_pool(name="outp", bufs=2))
            psum_mm = ctx.enter_context(tc.tile_pool(name="psum_mm", bufs=2, space="PSUM"))
            psum_tr = ctx.enter_context(tc.tile_pool(name="psum_tr", bufs=2, space="PSUM"))
            psum_o = ctx.enter_context(tc.tile_pool(name="psum_o", bufs=1, space="PSUM"))

            V = nc.vector
            for i in range(NT):
                tsl = bass.ts(i, P)
                xt = xin.tile([P, KT, P], bf, tag="xt")
                nc.sync.dma_start(
                    out=xt[:], in_=xT_d[:, tsl].rearrange("(k p) t -> p k t", p=P)
                )

                # ---- QKV projection: 3 passes of 2x512 cols, k-accumulated ----
                qkv = qkvp.tile([P, E3], bf, tag="qkv")
                for grp in range(3):
                    pa = psum_mm.tile([P, 512], f32, tag="mma")
                    pb = psum_mm.tile([P, 512], f32, tag="mmb")
                    for k in range(KT):
                        nc.tensor.matmul(
                            pa[:], xt[:, k, :], wq_sb[:, k, bass.ts(2 * grp, 512)],
                            start=(k == 0), stop=(k == KT - 1),
                        )
                        nc.tensor.matmul(
                            pb[:], xt[:, k, :], wq_sb[:, k, bass.ts(2 * grp + 1, 512)],
                            start=(k == 0), stop=(k == KT - 1),
                        )
                    nc.scalar.copy(qkv[:, bass.ts(2 * grp, 512)], pa[:])
                    nc.scalar.copy(qkv[:, bass.ts(2 * grp + 1, 512)], pb[:])

                # layouts: q [t,(h,d)], k [t,(g,d)] natural; v [t,(d,g)] d-major
                qv = qkv[:, 0:D].rearrange("p (h d) -> p h d", d=HD)
                kv = qkv[:, D:2 * D].rearrange("p (g d) -> p g d", d=HD)
                vdm = qkv[:, 2 * D:3 * D].rearrange("p (d g) -> p d g", g=H)

                # ---- scores products tmp[t,h,g,d] (one 2x op) ----
                tmp = work.tile([P, H, H, HD], bf, tag="tmp")
                q_b = qv[:, :, None, :].broadcast_to((P, H, H, HD))
                k_b = kv[:, None, :, :].broadcast_to((P, H, H, HD))
                V.tensor_mul(tmp[:], q_b, k_b)

                # ---- d-reduce: strided fold tree (all 2x) -> scores [t,(h,g)] ----
                b1 = work.tile([P, HH, 32], bf, tag="b1")
                b2 = work.tile([P, HH, 16], bf, tag="b2")
                scores = smp.tile([P, H, H], f32, tag="sc")
                tmpv = tmp.rearrange("p h g d -> p (h g) d")
                V.tensor_add(b1[:], tmpv[:, :, 0:32], tmpv[:, :, 32:64])
                V.tensor_add(b2[:], b1[:, :, 0:16], b1[:, :, 16:32])
                V.tensor_add(b1[:, :, 0:8], b2[:, :, 0:8], b2[:, :, 8:16])
                V.tensor_add(b2[:, :, 0:4], b1[:, :, 0:4], b1[:, :, 4:8])
                V.tensor_add(b1[:, :, 0:2], b2[:, :, 0:2], b2[:, :, 2:4])
                V.tensor_add(
                    scores.rearrange("p h g -> p (h g)")[:, :, None],
                    b1[:, :, 0:1], b1[:, :, 1:2],
                )

                # ---- softmax over g: we = exp(s/32); den; rec; wn = we*rec ----
                we = smp.tile([P, H, H], bf, tag="we")
                den = smp.tile([P, H], f32, tag="den")
                rec = smp.tile([P, H], f32, tag="rec")
                dn1 = smp.tile([P, H, 8], f32, tag="dn1")
                nc.scalar.activation(
                    we[:], scores[:], mybir.ActivationFunctionType.Exp, scale=1.0 / 32.0
                )
                V.tensor_add(dn1[:], we[:, :, 0:8], we[:, :, 8:16])
                V.tensor_add(dn1[:, :, 0:4], dn1[:, :, 0:4], dn1[:, :, 4:8])
                V.tensor_add(dn1[:, :, 0:2], dn1[:, :, 0:2], dn1[:, :, 2:4])
                V.tensor_add(den[:, :, None], dn1[:, :, 0:1], dn1[:, :, 1:2])
                V.reciprocal(rec[:], den[:])
                wn = smp.tile([P, H, H], bf, tag="wn")
                V.tensor_mul(wn[:], we[:], rec[:, :, None].broadcast_to((P, H, H)))

                # ---- attn products p2[t,h,d,g] = wn[t,h,g]*v[t,d,g] (2x) ----
                p2 = tmp.rearrange("p h g d -> p (h g d)").rearrange(
                    "p (h d g) -> p h d g", h=H, d=HD
                )
                wn_b = wn[:, :, None, :].broadcast_to((P, H, HD, H))
                v_b = vdm[:, None, :, :].broadcast_to((P, H, HD, H))
                V.tensor_mul(p2[:], wn_b, v_b)

                # ---- g-reduce: strided fold tree -> attn [t,(h,d)] bf16 ----
                c1 = b1.rearrange("p a x -> p (a x)").rearrange(
                    "p (a y) -> p a y", y=8
                )  # [P, 1024, 8]
                c2 = b2.rearrange("p a x -> p (a x)").rearrange(
                    "p (a y) -> p a y", y=4
                )  # [P, 1024, 4]
                p2v = p2.rearrange("p h d g -> p (h d) g")
                attn = attnp.tile([P, D], bf, tag="attn")
                V.tensor_add(c1[:], p2v[:, :, 0:8], p2v[:, :, 8:16])
                V.tensor_add(c2[:], c1[:, :, 0:4], c1[:, :, 4:8])
                V.tensor_add(c1[:, :, 0:2], c2[:, :, 0:2], c2[:, :, 2:4])
                V.tensor_add(
                    attn[:, :, None].rearrange("p a o -> p a o"),
                    c1[:, :, 0:1], c1[:, :, 1:2],
                )

                # ---- transpose attn -> attnT [(h d), t] blocks ----
                attnT = attnp.tile([P, KT, P], bf, tag="attnT")
                for c in range(KT):
                    pt = psum_tr.tile([P, P], bf, tag="pt")
                    nc.tensor.transpose(pt[:], attn[:, bass.ts(c, P)], ident[:])
                    nc.scalar.copy(attnT[:, c, :], pt[:])

                # ---- output projection ----
                outt = outp.tile([P, D], f32, tag="outt")
                po_a = psum_o.tile([P, 512], f32, tag="poa")
                po_b = psum_o.tile([P, 512], f32, tag="pob")
                for k in range(KT):
                    nc.tensor.matmul(
                        po_a[:], attnT[:, k, :], wo_sb[:, k, 0:512],
                        start=(k == 0), stop=(k == KT - 1),
                    )
                    nc.tensor.matmul(
                        po_b[:], attnT[:, k, :], wo_sb[:, k, 512:1024],
                        start=(k == 0), stop=(k == KT - 1),
                    )
                nc.scalar.copy(outt[:, 0:512], po_a[:])
                nc.scalar.copy(outt[:, 512:1024], po_b[:])
                nc.sync.dma_start(out=out_d[tsl, :], in_=outt[:])
    nc.compile()
    return nc, names


def kernel(x, Wqkv, Wo, bo, trace=False):
    if "nc" not in _CACHE:
        _CACHE["nc"], _CACHE["names"] = _build()
    nc, names = _CACHE["nc"], _CACHE["names"]
    bf = ml_dtypes.bfloat16
    xt = np.ascontiguousarray(
        np.asarray(x, dtype=np.float32).reshape(B * N, D).T
    )  # [D, B*N]
    wqkv = np.asarray(Wqkv, dtype=np.float32)
    # v rows permuted so the projection emits v[t, (d, g)] (d-major)
    wv = wqkv[2 * D:3 * D].reshape(H, HD, D).transpose(1, 0, 2).reshape(D, D)
    wqkv_perm = np.concatenate([wqkv[0:2 * D], wv], axis=0)
    wqkvT = np.ascontiguousarray(wqkv_perm.T).astype(bf)
    woT = np.ascontiguousarray(np.asarray(Wo, dtype=np.float32).T).astype(bf)
    in_maps = []
    for c in range(NCORES):
        shard = np.ascontiguousarray(xt[:, c * T:(c + 1) * T]).astype(bf)
        in_maps.append(
            {names["xT"]: shard, names["wqkvT"]: wqkvT, names["woT"]: woT}
        )
    res = run_bass_kernel_spmd(
        nc, in_maps, core_ids=list(range(NCORES)), trace=trace
    )
    shards = [res.results[c][names["out"]] for c in range(NCORES)]
    out = np.concatenate(shards, axis=0).reshape(B, N, D).astype(np.float32)
    out = out + np.asarray(bo, dtype=np.float32)[None, None, :]
    if trace:
        return out, res
    return out


# revision 7
# speedup vs baseline: 1.7332x; 1.0013x over previous
import sys
from contextlib import ExitStack

sys.path.insert(0, "/opt/trn_rl_repo")

import numpy as np
import ml_dtypes

import concourse.bass as bass
import concourse.bacc as bacc
import concourse.mybir as mybir
import concourse.tile as tile
from concourse.bass_utils import run_bass_kernel_spmd
from concourse.masks import make_identity

B, N, D, H, HD = 4, 4096, 1024, 16, 64
NCORES = 8
T = (B * N) // NCORES  # 2048 tokens per core
P = 128
NT = T // P            # 16 token tiles per core
KT = D // P            # 8 contraction tiles
E3 = 3 * D
HH = H * H             # 256

_CACHE = {}


def _name(t):
    return t.name if hasattr(t, "name") else t.tensor.name


def _build():
    bf = mybir.dt.bfloat16
    f32 = mybir.dt.float32
    nc = bacc.Bacc(None, target_bir_lowering=False)
    names = {}
    V = None
    with tile.TileContext(nc) as tc:
        with ExitStack() as ctx:
            dram = ctx.enter_context(tc.tile_pool(name="dram", bufs=1, space="DRAM"))
            xT_d = dram.tile([D, T], bf, kind="ExternalInput")
            wq_d = dram.tile([D, E3], bf, kind="ExternalInput")
            wo_d = dram.tile([D, D], bf, kind="ExternalInput")
            out_d = dram.tile([T, D], f32, kind="ExternalOutput")
            names["xT"] = _name(xT_d)
            names["wqkvT"] = _name(wq_d)
            names["woT"] = _name(wo_d)
            names["out"] = _name(out_d)

            consts = ctx.enter_context(tc.tile_pool(name="consts", bufs=1))
            wq_sb = consts.tile([P, KT, E3], bf)
            wo_sb = consts.tile([P, KT, D], bf)
            ident = consts.tile([P, P], bf)
            make_identity(nc, ident)
            for grp in range(6):
                nc.sync.dma_start(
                    out=wq_sb[:, :, bass.ts(grp, 512)],
                    in_=wq_d[:, bass.ts(grp, 512)].rearrange("(k p) e -> p k e", p=P),
                )
            nc.sync.dma_start(out=wo_sb[:], in_=wo_d[:].rearrange("(k p) e -> p k e", p=P))

            xin = ctx.enter_context(tc.tile_pool(name="xin", bufs=2))
            qkvp = ctx.enter_context(tc.tile_pool(name="qkvp", bufs=2))
            work = ctx.enter_context(tc.tile_pool(name="work", bufs=1))
            smp = ctx.enter_context(tc.tile_pool(name="smp", bufs=1))
            attnp = ctx.enter_context(tc.tile_pool(name="attnp", bufs=2))
            outp = ctx.enter_context(tc.tile_pool(name="outp", bufs=2))
            psum_mm = ctx.enter_context(tc.tile_pool(name="psum_mm", bufs=2, space="PSUM"))
            psum_tr = ctx.enter_context(tc.tile_pool(name="psum_tr", bufs=2, space="PSUM"))
            psum_o = ctx.enter_context(tc.tile_pool(name="psum_o", bufs=1, space="PSUM"))

            V = nc.vector
            for i in range(NT):
                tsl = bass.ts(i, P)
                xt = xin.tile([P, KT, P], bf, tag="xt")
                nc.sync.dma_start(
                    out=xt[:], in_=xT_d[:, tsl].rearrange("(k p) t -> p k t", p=P)
                )

                # ---- QKV projection: 3 passes of 2x512 cols, k-accumulated ----
                qkv = qkvp.tile([P, E3], bf, tag="qkv")
                for grp in range(3):
                    pa = psum_mm.tile([P, 512], f32, tag="mma")
                    pb = psum_mm.tile([P, 512], f32, tag="mmb")
                    for k in range(KT):
                        nc.tensor.matmul(
                            pa[:], xt[:, k, :], wq_sb[:, k, bass.ts(2 * grp, 512)],
                            start=(k == 0), stop=(k == KT - 1),
                        )
                        nc.tensor.matmul(
                            pb[:], xt[:, k, :], wq_sb[:, k, bass.ts(2 * grp + 1, 512)],
                            start=(k == 0), stop=(k == KT - 1),
                        )
                    nc.scalar.copy(qkv[:, bass.ts(2 * grp, 512)], pa[:])
                    nc.scalar.copy(qkv[:, bass.ts(2 * grp + 1, 512)], pb[:])

                # layouts: q [t,(h,d)], k [t,(g,d)] natural; v [t,(d,g)] d-major
                qv = qkv[:, 0:D].rearrange("p (h d) -> p h d", d=HD)
                kv = qkv[:, D:2 * D].rearrange("p (g d) -> p g d", d=HD)
                vdm = qkv[:, 2 * D:3 * D].rearrange("p (d g) -> p d g", g=H)

                # ---- scores products tmp[t,h,g,d] (one 2x op) ----
                tmp = work.tile([P, H, H, HD], bf, tag="tmp")
                q_b = qv[:, :, None, :].broadcast_to((P, H, H, HD))
                k_b = kv[:, None, :, :].broadcast_to((P, H, H, HD))
                V.tensor_mul(tmp[:], q_b, k_b)

                # ---- d-reduce: strided fold tree (all 2x) -> scores [t,(h,g)] ----
                b1 = work.tile([P, HH, 32], bf, tag="b1")
                b2 = work.tile([P, HH, 16], bf, tag="b2")
                scores = smp.tile([P, H, H], bf, tag="sc")
                tmpv = tmp.rearrange("p h g d -> p (h g) d")
                V.tensor_add(b1[:], tmpv[:, :, 0:32], tmpv[:, :, 32:64])
                V.tensor_add(b2[:], b1[:, :, 0:16], b1[:, :, 16:32])
                V.tensor_add(b1[:, :, 0:8], b2[:, :, 0:8], b2[:, :, 8:16])
                V.tensor_add(b2[:, :, 0:4], b1[:, :, 0:4], b1[:, :, 4:8])
                V.tensor_add(b1[:, :, 0:2], b2[:, :, 0:2], b2[:, :, 2:4])
                V.tensor_add(
                    scores.rearrange("p h g -> p (h g)")[:, :, None],
                    b1[:, :, 0:1], b1[:, :, 1:2],
                )

                # ---- softmax over g: we = exp(s/32); den; rec; wn = we*rec ----
                we = smp.tile([P, H, H], bf, tag="we")
                den = smp.tile([P, H], f32, tag="den")
                rec = smp.tile([P, H], f32, tag="rec")
                dn1 = smp.tile([P, H, 8], f32, tag="dn1")
                nc.scalar.activation(
                    we[:], scores[:], mybir.ActivationFunctionType.Exp, scale=1.0 / 32.0
                )
                V.tensor_add(dn1[:], we[:, :, 0:8], we[:, :, 8:16])
                V.tensor_add(dn1[:, :, 0:4], dn1[:, :, 0:4], dn1[:, :, 4:8])
                V.tensor_add(dn1[:, :, 0:2], dn1[:, :, 0:2], dn1[:, :, 2:4])
                V.tensor_add(den[:, :, None], dn1[:, :, 0:1], dn1[:, :, 1:2])
                V.reciprocal(rec[:], den[:])
                wn = smp.tile([P, H, H], bf, tag="wn")
                V.tensor_mul(wn[:], we[:], rec[:, :, None].broadcast_to((P, H, H)))

                # ---- attn products p2[t,h,d,g] = wn[t,h,g]*v[t,d,g] (2x) ----
                p2 = tmp.rearrange("p h g d -> p (h g d)").rearrange(
                    "p (h d g) -> p h d g", h=H, d=HD
                )
                wn_b = wn[:, :, None, :].broadcast_to((P, H, HD, H))
                v_b = vdm[:, None, :, :].broadcast_to((P, H, HD, H))
                V.tensor_mul(p2[:], wn_b, v_b)

                # ---- g-reduce: strided fold tree -> attn [t,(h,d)] bf16 ----
                c1 = b1.rearrange("p a x -> p (a x)").rearrange(
                    "p (a y) -> p a y", y=8
                )  # [P, 1024, 8]
                c2 = b2.rearrange("p a x -> p (a x)").rearrange(
                    "p (a y) -> p a y", y=4
                )  # [P, 1024, 4]
                p2v = p2.rearrange("p h d g -> p (h d) g")
                attn = attnp.tile([P, D], bf, tag="attn")
                V.tensor_add(c1[:], p2v[:, :, 0:8], p2v[:, :, 8:16])
                V.tensor_add(c2[:], c1[:, :, 0:4], c1[:, :, 4:8])
                V.tensor_add(c1[:, :, 0:2], c2[:, :, 0:2], c2[:, :, 2:4])
                V.tensor_add(
                    attn[:, :, None].rearrange("p a o -> p a o"),
                    c1[:, :, 0:1], c1[:, :, 1:2],
                )

                # ---- transpose attn -> attnT [(h d), t] blocks ----
                attnT = attnp.tile([P, KT, P], bf, tag="attnT")
                for c in range(KT):
                    pt = psum_tr.tile([P, P], bf, tag="pt")
                    nc.tensor.transpose(pt[:], attn[:, bass.ts(c, P)], ident[:])
                    nc.scalar.copy(attnT[:, c, :], pt[:])

                # ---- output projection ----
                outt = outp.tile([P, D], f32, tag="outt")
                po_a = psum_o.tile([P, 512], f32, tag="poa")
                po_b = psum_o.tile([P, 512], f32, tag="pob")
                for k in range(KT):
                    nc.tensor.matmul(
                        po_a[:], attnT[:, k, :], wo_sb[:, k, 0:512],
                        start=(k == 0), stop=(k == KT - 1),
                    )
                    nc.tensor.matmul(
                        po_b[:], attnT[:, k, :], wo_sb[:, k, 512:1024],
                        start=(k == 0), stop=(k == KT - 1),
                    )
                nc.scalar.copy(outt[:, 0:512], po_a[:])
                nc.scalar.copy(outt[:, 512:1024], po_b[:])
                nc.sync.dma_start(out=out_d[tsl, :], in_=outt[:])
    nc.compile()
    return nc, names


def kernel(x, Wqkv, Wo, bo, trace=False):
    if "nc" not in _CACHE:
        _CACHE["nc"], _CACHE["names"] = _build()
    nc, names = _CACHE["nc"], _CACHE["names"]
    bf = ml_dtypes.bfloat16
    xt = np.ascontiguousarray(
        np.asarray(x, dtype=np.float32).reshape(B * N, D).T
    )  # [D, B*N]
    wqkv = np.asarray(Wqkv, dtype=np.float32)
    # v rows permuted so the projection emits v[t, (d, g)] (d-major)
    wv = wqkv[2 * D:3 * D].reshape(H, HD, D).transpose(1, 0, 2).reshape(D, D)
    wqkv_perm = np.concatenate([wqkv[0:2 * D], wv], axis=0)
    wqkvT = np.ascontiguousarray(wqkv_perm.T).astype(bf)
    woT = np.ascontiguousarray(np.asarray(Wo, dtype=np.float32).T).astype(bf)
    in_maps = []
    for c in range(NCORES):
        shard = np.ascontiguousarray(xt[:, c * T:(c + 1) * T]).astype(bf)
        in_maps.append(
            {names["xT"]: shard, names["wqkvT"]: wqkvT, names["woT"]: woT}
        )
    res = run_bass_kernel_spmd(
        nc, in_maps, core_ids=list(range(NCORES)), trace=trace
    )
    shards = [res.results[c][names["out"]] for c in range(NCORES)]
    out = np.concatenate(shards, axis=0).reshape(B, N, D).astype(np.float32)
    out = out + np.asarray(bo, dtype=np.float32)[None, None, :]
    if trace:
        return out, res
    return out


# revision 10
# speedup vs baseline: 1.7863x; 1.0307x over previous
import sys
from contextlib import ExitStack

sys.path.insert(0, "/opt/trn_rl_repo")

import numpy as np
import ml_dtypes

import concourse.bass as bass
import concourse.bacc as bacc
import concourse.mybir as mybir
import concourse.tile as tile
from concourse.bass_utils import run_bass_kernel_spmd
from concourse.masks import make_identity

B, N, D, H, HD = 4, 4096, 1024, 16, 64
NCORES = 8
T = (B * N) // NCORES  # 2048 tokens per core
P = 128
NT = T // P            # 16 token tiles per core
KT = D // P            # 8 contraction tiles
E3 = 3 * D
HH = H * H             # 256

_CACHE = {}


def _name(t):
    return t.name if hasattr(t, "name") else t.tensor.name


def _build():
    bf = mybir.dt.bfloat16
    f32 = mybir.dt.float32
    nc = bacc.Bacc(None, target_bir_lowering=False)
    names = {}
    V = None
    with tile.TileContext(nc) as tc:
        with ExitStack() as ctx:
            dram = ctx.enter_context(tc.tile_pool(name="dram", bufs=1, space="DRAM"))
            xT_d = dram.tile([D, T], bf, kind="ExternalInput")
            wq_d = dram.tile([D, E3], bf, kind="ExternalInput")
            wo_d = dram.tile([D, D], bf, kind="ExternalInput")
            out_d = dram.tile([T, D], f32, kind="ExternalOutput")
            names["xT"] = _name(xT_d)
            names["wqkvT"] = _name(wq_d)
            names["woT"] = _name(wo_d)
            names["out"] = _name(out_d)

            consts = ctx.enter_context(tc.tile_pool(name="consts", bufs=1))
            wq_sb = consts.tile([P, KT, E3], bf)
            wo_sb = consts.tile([P, KT, D], bf)
            ident = consts.tile([P, P], bf)
            make_identity(nc, ident)
            xin = ctx.enter_context(tc.tile_pool(name="xin", bufs=2))
            xt0 = xin.tile([P, KT, P], bf, tag="xt")
            nc.sync.dma_start(
                out=xt0[:, 0:4], in_=xT_d[0:512, 0:P].rearrange("(k p) t -> p k t", p=P)
            )
            nc.sync.dma_start(
                out=xt0[:, 4:8], in_=xT_d[512:1024, 0:P].rearrange("(k p) t -> p k t", p=P)
            )
            for grp in range(6):
                nc.sync.dma_start(
                    out=wq_sb[:, :, bass.ts(grp, 512)],
                    in_=wq_d[:, bass.ts(grp, 512)].rearrange("(k p) e -> p k e", p=P),
                )
            nc.sync.dma_start(out=wo_sb[:], in_=wo_d[:].rearrange("(k p) e -> p k e", p=P))

            qkvp = ctx.enter_context(tc.tile_pool(name="qkvp", bufs=2))
            work = ctx.enter_context(tc.tile_pool(name="work", bufs=1))
            smp = ctx.enter_context(tc.tile_pool(name="smp", bufs=1))
            attnp = ctx.enter_context(tc.tile_pool(name="attnp", bufs=2))
            outp = ctx.enter_context(tc.tile_pool(name="outp", bufs=2))
            psum_mm = ctx.enter_context(tc.tile_pool(name="psum_mm", bufs=2, space="PSUM"))
            psum_tr = ctx.enter_context(tc.tile_pool(name="psum_tr", bufs=2, space="PSUM"))
            psum_o = ctx.enter_context(tc.tile_pool(name="psum_o", bufs=1, space="PSUM"))

            V = nc.vector
            for i in range(NT):
                tsl = bass.ts(i, P)
                if i == 0:
                    xt = xt0
                else:
                    xt = xin.tile([P, KT, P], bf, tag="xt")
                    nc.sync.dma_start(
                        out=xt[:], in_=xT_d[:, tsl].rearrange("(k p) t -> p k t", p=P)
                    )

                # ---- QKV projection: 3 passes of 2x512 cols, k-accumulated ----
                qkv = qkvp.tile([P, E3], bf, tag="qkv")
                for grp in range(3):
                    pa = psum_mm.tile([P, 512], f32, tag="mma")
                    pb = psum_mm.tile([P, 512], f32, tag="mmb")
                    for k in range(KT):
                        nc.tensor.matmul(
                            pa[:], xt[:, k, :], wq_sb[:, k, bass.ts(2 * grp, 512)],
                            start=(k == 0), stop=(k == KT - 1),
                        )
                        nc.tensor.matmul(
                            pb[:], xt[:, k, :], wq_sb[:, k, bass.ts(2 * grp + 1, 512)],
                            start=(k == 0), stop=(k == KT - 1),
                        )
                    nc.scalar.copy(qkv[:, bass.ts(2 * grp, 512)], pa[:])
                    nc.scalar.copy(qkv[:, bass.ts(2 * grp + 1, 512)], pb[:])

                # layouts: q [t,(h,d)], k [t,(g,d)] natural; v [t,(d,g)] d-major
                qv = qkv[:, 0:D].rearrange("p (h d) -> p h d", d=HD)
                kv = qkv[:, D:2 * D].rearrange("p (g d) -> p g d", d=HD)
                vdm = qkv[:, 2 * D:3 * D].rearrange("p (d g) -> p d g", g=H)

                # ---- scores products tmp[t,h,g,d] (one 2x op) ----
                tmp = work.tile([P, H, H, HD], bf, tag="tmp")
                q_b = qv[:, :, None, :].broadcast_to((P, H, H, HD))
                k_b = kv[:, None, :, :].broadcast_to((P, H, H, HD))
                V.tensor_mul(tmp[:], q_b, k_b)

                # ---- d-reduce: strided fold tree (all 2x) -> scores [t,(h,g)] ----
                b1 = work.tile([P, HH, 32], bf, tag="b1")
                b2 = work.tile([P, HH, 16], bf, tag="b2")
                scores = smp.tile([P, H, H], bf, tag="sc")
                tmpv = tmp.rearrange("p h g d -> p (h g) d")
                V.tensor_add(b1[:], tmpv[:, :, 0:32], tmpv[:, :, 32:64])
                V.tensor_add(b2[:], b1[:, :, 0:16], b1[:, :, 16:32])
                V.tensor_add(b1[:, :, 0:8], b2[:, :, 0:8], b2[:, :, 8:16])
                V.tensor_add(b2[:, :, 0:4], b1[:, :, 0:4], b1[:, :, 4:8])
                V.tensor_add(b1[:, :, 0:2], b2[:, :, 0:2], b2[:, :, 2:4])
                V.tensor_add(
                    scores.rearrange("p h g -> p (h g)")[:, :, None],
                    b1[:, :, 0:1], b1[:, :, 1:2],
                )

                # ---- softmax over g: we = exp(s/32); den; rec; wn = we*rec ----
                we = smp.tile([P, H, H], bf, tag="we")
                den = smp.tile([P, H], f32, tag="den")
                rec = smp.tile([P, H], f32, tag="rec")
                dn1 = smp.tile([P, H, 8], f32, tag="dn1")
                nc.scalar.activation(
                    we[:], scores[:], mybir.ActivationFunctionType.Exp, scale=1.0 / 32.0
                )
                V.tensor_add(dn1[:], we[:, :, 0:8], we[:, :, 8:16])
                V.tensor_add(dn1[:, :, 0:4], dn1[:, :, 0:4], dn1[:, :, 4:8])
                V.tensor_add(dn1[:, :, 0:2], dn1[:, :, 0:2], dn1[:, :, 2:4])
                V.tensor_add(den[:, :, None], dn1[:, :, 0:1], dn1[:, :, 1:2])
                V.reciprocal(rec[:], den[:])
                wn = smp.tile([P, H, H], bf, tag="wn")
                V.tensor_mul(wn[:], we[:], rec[:, :, None].broadcast_to((P, H, H)))

                # ---- attn products p2[t,h,d,g] = wn[t,h,g]*v[t,d,g] (2x) ----
                p2 = tmp.rearrange("p h g d -> p (h g d)").rearrange(
                    "p (h d g) -> p h d g", h=H, d=HD
                )
                wn_b = wn[:, :, None, :].broadcast_to((P, H, HD, H))
                v_b = vdm[:, None, :, :].broadcast_to((P, H, HD, H))
                V.tensor_mul(p2[:], wn_b, v_b)

                # ---- g-reduce: strided fold tree -> attn [t,(h,d)] bf16 ----
                c1 = b1.rearrange("p a x -> p (a x)").rearrange(
                    "p (a y) -> p a y", y=8
                )  # [P, 1024, 8]
                c2 = b2.rearrange("p a x -> p (a x)").rearrange(
                    "p (a y) -> p a y", y=4
                )  # [P, 1024, 4]
                p2v = p2.rearrange("p h d g -> p (h d) g")
                attn = attnp.tile([P, D], bf, tag="attn")
                V.tensor_add(c1[:], p2v[:, :, 0:8], p2v[:, :, 8:16])
                V.tensor_add(c2[:], c1[:, :, 0:4], c1[:, :, 4:8])
                V.tensor_add(c1[:, :, 0:2], c2[:, :, 0:2], c2[:, :, 2:4])
                V.tensor_add(
                    attn[:, :, None].rearrange("p a o -> p a o"),
                    c1[:, :, 0:1], c1[:, :, 1:2],
                )

                # ---- transpose attn -> attnT [(h d), t] blocks ----
                attnT = attnp.tile([P, KT, P], bf, tag="attnT")
                for c in range(KT):
                    pt = psum_tr.tile([P, P], bf, tag="pt")
                    nc.tensor.transpose(pt[:], attn[:, bass.ts(c, P)], ident[:])
                    nc.scalar.copy(attnT[:, c, :], pt[:])

                # ---- output projection ----
                outt = outp.tile([P, D], f32, tag="outt")
                po_a = psum_o.tile([P, 512], f32, tag="poa")
                po_b = psum_o.tile([P, 512], f32, tag="pob")
                for k in range(KT):
                    nc.tensor.matmul(
                        po_a[:], attnT[:, k, :], wo_sb[:, k, 0:512],
                        start=(k == 0), stop=(k == KT - 1),
                    )
                    nc.tensor.matmul(
                        po_b[:], attnT[:, k, :], wo_sb[:, k, 512:1024],
                        start=(k == 0), stop=(k == KT - 1),
                    )
                nc.scalar.copy(outt[:, 0:512], po_a[:])
                nc.scalar.copy(outt[:, 512:1024], po_b[:])
                nc.sync.dma_start(out=out_d[tsl, :], in_=outt[:])
    nc.compile()
    return nc, names


def kernel(x, Wqkv, Wo, bo, trace=False):
    if "nc" not in _CACHE:
        _CACHE["nc"], _CACHE["names"] = _build()
    nc, names = _CACHE["nc"], _CACHE["names"]
    bf = ml_dtypes.bfloat16
    xt = np.ascontiguousarray(
        np.asarray(x, dtype=np.float32).reshape(B * N, D).T
    )  # [D, B*N]
    wqkv = np.asarray(Wqkv, dtype=np.float32)
    # v rows permuted so the projection emits v[t, (d, g)] (d-major)
    wv = wqkv[2 * D:3 * D].reshape(H, HD, D).transpose(1, 0, 2).reshape(D, D)
    wqkv_perm = np.concatenate([wqkv[0:2 * D], wv], axis=0)
    wqkvT = np.ascontiguousarray(wqkv_perm.T).astype(bf)
    woT = np.ascontiguousarray(np.asarray(Wo, dtype=np.float32).T).astype(bf)
    in_maps = []
    for c in range(NCORES):
        shard = np.ascontiguousarray(xt[:, c * T:(c + 1) * T]).astype(bf)
        in_maps.append(
            {names["xT"]: shard, names["wqkvT"]: wqkvT, names["woT"]: woT}
        )
    res = run_bass_kernel_spmd(
        nc, in_maps, core_ids=list(range(NCORES)), trace=trace
    )
    shards = [res.results[c][names["out"]] for c in range(NCORES)]
    out = np.concatenate(shards, axis=0).reshape(B, N, D).astype(np.float32)
    out = out + np.asarray(bo, dtype=np.float32)[None, None, :]
    if trace:
        return out, res
    return out


# revision 12
# speedup vs baseline: 1.9261x; 1.0782x over previous
import sys
from contextlib import ExitStack

sys.path.insert(0, "/opt/trn_rl_repo")

import numpy as np
import ml_dtypes

import concourse.bass as bass
import concourse.bacc as bacc
import concourse.mybir as mybir
import concourse.tile as tile
from concourse.bass_utils import run_bass_kernel_spmd
from concourse.masks import make_identity

B, N, D, H, HD = 4, 4096, 1024, 16, 64
NCORES = 8
T = (B * N) // NCORES  # 2048 tokens per core
P = 128
NT = T // P            # 16 token tiles per core
KT = D // P            # 8 contraction tiles
E3 = 3 * D
HH = H * H             # 256

_CACHE = {}


def _name(t):
    return t.name if hasattr(t, "name") else t.tensor.name


def _build():
    bf = mybir.dt.bfloat16
    f32 = mybir.dt.float32
    nc = bacc.Bacc(None, target_bir_lowering=False)
    names = {}
    V = None
    with tile.TileContext(nc) as tc:
        with ExitStack() as ctx:
            dram = ctx.enter_context(tc.tile_pool(name="dram", bufs=1, space="DRAM"))
            xT_d = dram.tile([D, T], bf, kind="ExternalInput")
            wq_d = dram.tile([D, E3], bf, kind="ExternalInput")
            wo_d = dram.tile([D, D], bf, kind="ExternalInput")
            out_d = dram.tile([T, D], f32, kind="ExternalOutput")
            names["xT"] = _name(xT_d)
            names["wqkvT"] = _name(wq_d)
            names["woT"] = _name(wo_d)
            names["out"] = _name(out_d)

            consts = ctx.enter_context(tc.tile_pool(name="consts", bufs=1))
            wq_sb = consts.tile([P, KT, E3], bf)
            wo_sb = consts.tile([P, KT, D], bf)
            ident = consts.tile([P, P], bf)
            make_identity(nc, ident)
            xin = ctx.enter_context(tc.tile_pool(name="xin", bufs=2))
            xt0 = xin.tile([P, KT, P], bf, tag="xt")
            nc.sync.dma_start(
                out=xt0[:, 0:4], in_=xT_d[0:512, 0:P].rearrange("(k p) t -> p k t", p=P)
            )
            nc.sync.dma_start(
                out=xt0[:, 4:8], in_=xT_d[512:1024, 0:P].rearrange("(k p) t -> p k t", p=P)
            )
            for grp in range(6):
                nc.sync.dma_start(
                    out=wq_sb[:, :, bass.ts(grp, 512)],
                    in_=wq_d[:, bass.ts(grp, 512)].rearrange("(k p) e -> p k e", p=P),
                )
            nc.sync.dma_start(out=wo_sb[:], in_=wo_d[:].rearrange("(k p) e -> p k e", p=P))

            qkvp = ctx.enter_context(tc.tile_pool(name="qkvp", bufs=2))
            work = ctx.enter_context(tc.tile_pool(name="work", bufs=1))
            smp = ctx.enter_context(tc.tile_pool(name="smp", bufs=1))
            attnp = ctx.enter_context(tc.tile_pool(name="attnp", bufs=2))
            outp = ctx.enter_context(tc.tile_pool(name="outp", bufs=2))
            psum_mm = ctx.enter_context(tc.tile_pool(name="psum_mm", bufs=2, space="PSUM"))
            psum_tr = ctx.enter_context(tc.tile_pool(name="psum_tr", bufs=2, space="PSUM"))
            psum_o = ctx.enter_context(tc.tile_pool(name="psum_o", bufs=1, space="PSUM"))

            V = nc.vector
            for i in range(NT):
                tsl = bass.ts(i, P)
                if i == 0:
                    xt = xt0
                else:
                    xt = xin.tile([P, KT, P], bf, tag="xt")
                    nc.sync.dma_start(
                        out=xt[:], in_=xT_d[:, tsl].rearrange("(k p) t -> p k t", p=P)
                    )

                # ---- QKV projection: 3 passes of 2x512 cols, k-accumulated ----
                qkv = qkvp.tile([P, E3], bf, tag="qkv")
                for grp in range(3):
                    pa = psum_mm.tile([P, 512], f32, tag="mma")
                    pb = psum_mm.tile([P, 512], f32, tag="mmb")
                    for k in range(KT):
                        nc.tensor.matmul(
                            pa[:], xt[:, k, :], wq_sb[:, k, bass.ts(2 * grp, 512)],
                            start=(k == 0), stop=(k == KT - 1),
                        )
                        nc.tensor.matmul(
                            pb[:], xt[:, k, :], wq_sb[:, k, bass.ts(2 * grp + 1, 512)],
                            start=(k == 0), stop=(k == KT - 1),
                        )
                    nc.scalar.copy(qkv[:, bass.ts(2 * grp, 512)], pa[:])
                    nc.scalar.copy(qkv[:, bass.ts(2 * grp + 1, 512)], pb[:])

                # layouts: q [t,(h,d)], k [t,(g,d)] natural; v [t,(d,g)] d-major
                qv = qkv[:, 0:D].rearrange("p (h d) -> p h d", d=HD)
                kv = qkv[:, D:2 * D].rearrange("p (g d) -> p g d", d=HD)
                vdm = qkv[:, 2 * D:3 * D].rearrange("p (d g) -> p d g", g=H)

                # ---- scores products tmp[t,h,g,d] (one 2x op) ----
                tmp = work.tile([P, H, H, HD], bf, tag="tmp")
                q_b = qv[:, :, None, :].broadcast_to((P, H, H, HD))
                k_b = kv[:, None, :, :].broadcast_to((P, H, H, HD))
                V.tensor_mul(tmp[:], q_b, k_b)

                # ---- d-reduce: strided fold tree (all 2x) -> scores [t,(h,g)] ----
                b1 = work.tile([P, HH, 32], bf, tag="b1")
                b2 = work.tile([P, HH, 16], bf, tag="b2")
                scores = smp.tile([P, H, H], bf, tag="sc")
                tmpv = tmp.rearrange("p h g d -> p (h g) d")
                V.tensor_add(b1[:], tmpv[:, :, 0:32], tmpv[:, :, 32:64])
                V.tensor_add(b2[:], b1[:, :, 0:16], b1[:, :, 16:32])
                V.tensor_add(b1[:, :, 0:8], b2[:, :, 0:8], b2[:, :, 8:16])
                V.tensor_add(b2[:, :, 0:4], b1[:, :, 0:4], b1[:, :, 4:8])
                V.tensor_add(b1[:, :, 0:2], b2[:, :, 0:2], b2[:, :, 2:4])
                V.tensor_add(
                    scores.rearrange("p h g -> p (h g)")[:, :, None],
                    b1[:, :, 0:1], b1[:, :, 1:2],
                )

                # ---- softmax over g: we = exp(s/32); den; rec; wn = we*rec ----
                we = smp.tile([P, H, H], bf, tag="we")
                den = smp.tile([P, H], f32, tag="den")
                rec = smp.tile([P, H], f32, tag="rec")
                dn1 = smp.tile([P, H, 8], f32, tag="dn1")
                nc.scalar.activation(
                    we[:], scores[:], mybir.ActivationFunctionType.Exp, scale=1.0 / 32.0
                )
                V.tensor_add(dn1[:], we[:, :, 0:8], we[:, :, 8:16])
                V.tensor_add(dn1[:, :, 0:4], dn1[:, :, 0:4], dn1[:, :, 4:8])
                V.reduce_sum(den[:, :, None], dn1[:, :, 0:4], axis=mybir.AxisListType.X)
                V.reciprocal(rec[:], den[:])
                wn = smp.tile([P, H, H], bf, tag="wn")
                V.tensor_mul(wn[:], we[:], rec[:, :, None].broadcast_to((P, H, H)))

                # ---- attn products p2[t,h,d,g] = wn[t,h,g]*v[t,d,g] (2x) ----
                p2 = tmp.rearrange("p h g d -> p (h g d)").rearrange(
                    "p (h d g) -> p h d g", h=H, d=HD
                )
                wn_b = wn[:, :, None, :].broadcast_to((P, H, HD, H))
                v_b = vdm[:, None, :, :].broadcast_to((P, H, HD, H))
                V.tensor_mul(p2[:], wn_b, v_b)

                # ---- g-reduce: fold only 16->4; PSUM eats the remnant ----
                c1 = b1.rearrange("p a x -> p (a x)").rearrange(
                    "p (a y) -> p a y", y=8
                )  # [P, 1024, 8]
                p2v = p2.rearrange("p h d g -> p (h d) g")
                attn4 = attnp.tile([P, D, 4], bf, tag="attn4")
                V.tensor_add(c1[:], p2v[:, :, 0:8], p2v[:, :, 8:16])
                V.tensor_add(attn4[:], c1[:, :, 0:4], c1[:, :, 4:8])

                # ---- transpose attn4 per g-remnant -> attnT [(h d), t] ----
                attnT = attnp.tile([P, 4 * KT, P], bf, tag="attnT")
                for gr in range(4):
                    for c in range(KT):
                        pt = psum_tr.tile([P, P], bf, tag="pt")
                        nc.tensor.transpose(
                            pt[:], attn4[:, bass.ts(c, P), gr], ident[:]
                        )
                        nc.scalar.copy(attnT[:, gr * KT + c, :], pt[:])

                # ---- output projection: accumulate over (gr, k) ----
                outt = outp.tile([P, D], f32, tag="outt")
                po_a = psum_o.tile([P, 512], f32, tag="poa")
                po_b = psum_o.tile([P, 512], f32, tag="pob")
                for gr in range(4):
                    for k in range(KT):
                        st = gr == 0 and k == 0
                        sp = gr == 3 and k == KT - 1
                        nc.tensor.matmul(
                            po_a[:], attnT[:, gr * KT + k, :], wo_sb[:, k, 0:512],
                            start=st, stop=sp,
                        )
                        nc.tensor.matmul(
                            po_b[:], attnT[:, gr * KT + k, :], wo_sb[:, k, 512:1024],
                            start=st, stop=sp,
                        )
                nc.scalar.copy(outt[:, 0:512], po_a[:])
                nc.scalar.copy(outt[:, 512:1024], po_b[:])
                nc.sync.dma_start(out=out_d[tsl, :], in_=outt[:])
    nc.compile()
    return nc, names


def kernel(x, Wqkv, Wo, bo, trace=False):
    if "nc" not in _CACHE:
        _CACHE["nc"], _CACHE["names"] = _build()
    nc, names = _CACHE["nc"], _CACHE["names"]
    bf = ml_dtypes.bfloat16
    xt = np.ascontiguousarray(
        np.asarray(x, dtype=np.float32).reshape(B * N, D).T
    )  # [D, B*N]
    wqkv = np.asarray(Wqkv, dtype=np.float32)
    # v rows permuted so the projection emits v[t, (d, g)] (d-major)
    wv = wqkv[2 * D:3 * D].reshape(H, HD, D).transpose(1, 0, 2).reshape(D, D)
    wqkv_perm = np.concatenate([wqkv[0:2 * D], wv], axis=0)
    wqkvT = np.ascontiguousarray(wqkv_perm.T).astype(bf)
    woT = np.ascontiguousarray(np.asarray(Wo, dtype=np.float32).T).astype(bf)
    in_maps = []
    for c in range(NCORES):
        shard = np.ascontiguousarray(xt[:, c * T:(c + 1) * T]).astype(bf)
        in_maps.append(
            {names["xT"]: shard, names["wqkvT"]: wqkvT, names["woT"]: woT}
        )
    res = run_bass_kernel_spmd(
        nc, in_maps, core_ids=list(range(NCORES)), trace=trace
    )
    shards = [res.results[c][names["out"]] for c in range(NCORES)]
    out = np.concatenate(shards, axis=0).reshape(B, N, D).astype(np.float32)
    out = out + np.asarray(bo, dtype=np.float32)[None, None, :]
    if trace:
        return out, res
    return out
